# revision 10
# baseline (speedup 1.0000x reference)
"""Two-layer LSTM (B=256, T=128, F=128, H=1024) + output projection on 8 TRN2 NeuronCores.

Sharding: pure data-parallel over batch (32 rows/core), weights replicated.
Per-core recurrent matmuls run with the small batch dim as the stationary
free dim (M=32) using 4-way TensorEngine column tiling (tile_position) so
four N=512 gate-chunk streams execute concurrently (~60% PE utilization
vs 25% untiled). All matmul operands are bf16 (fp32 PSUM accumulation);
cell math is fp32 on ACT/DVE. h-state transposes ride the DMA xbar
(bf16). The layer-0 input projection (x@Wih0^T + b0) is precomputed
on-device for all timesteps at full 128-row PE utilization in a scoped
SBUF pool, then streamed back per step. The output projection
accumulates into a pinned PSUM bank across all 128 timesteps, so there
is no tail matmul phase.
"""

import numpy as np
import ml_dtypes

B, T, F, H, O = 256, 128, 128, 1024, 128
NCORES = 8
BL = B // NCORES  # 32
KH = H // 128     # 8 contraction chunks
G4 = 4 * H        # 4096

_cache = {}
_DUMP = False


def _build():
    import concourse.bass as bass
    import concourse.tile as tile
    import concourse.mybir as mybir
    from concourse import bacc
    from concourse.bass import ds

    F32, BF16 = mybir.dt.float32, mybir.dt.bfloat16
    AF = mybir.ActivationFunctionType
    MULT, ADD = mybir.AluOpType.mult, mybir.AluOpType.add

    nc = bacc.Bacc("TRN2", target_bir_lowering=False, debug=False,
                   num_devices=NCORES, dynamic_dma_scratch_size=4096)

    def din(name, shape, dt):
        return nc.dram_tensor(name, shape, dt, kind="ExternalInput").ap()

    xT_d = din("xT", [128, T * BL], BF16)
    whh0T_d = din("whh0T", [128, KH * G4], BF16)
    wih1T_d = din("wih1T", [128, KH * G4], BF16)
    whh1T_d = din("whh1T", [128, KH * G4], BF16)
    wih0T_d = din("wih0T", [128, G4], BF16)
    b0r_d = din("b0r", [128, G4], BF16)     # b0 replicated across rows
    b1q_d = din("b1q", [128, 1024], BF16)   # b1 in quadrant layout
    woS_d = din("woS", [256, 128, O], BF16)
    h0T_d = din("h0T", [128, KH * BL], BF16)
    h1T_d = din("h1T", [128, KH * BL], BF16)
    c0_d = din("c0", [64, 512], F32)
    c1_d = din("c1", [64, 512], F32)
    out_d = nc.dram_tensor("out", [T, O], F32, kind="ExternalOutput").ap()
    if _DUMP:
        dmp = {nm: nc.dram_tensor(f"dmp_{nm}", shp, dt, kind="ExternalOutput").ap()
               for nm, shp, dt in [
                   ("h0T", [128, KH * BL], BF16), ("h1T", [128, KH * BL], BF16),
                   ("c0", [64, 512], F32), ("c1", [64, 512], F32),
                   ("xp", [128, 1024], BF16)]}

    with tile.TileContext(nc) as tc:
        with tc.tile_pool(name="dram", bufs=1, space="DRAM") as dp:
            # xp[t, bank, part=(q,b), j] : per-step gate quadrant layout
            xp_d = dp.tile([T, 2, 128, 512], BF16)
            # h1 history, flat A-row layout: hist[t, b, qt, j] = bf16(h1(t))[b, 512*qt+j]
            hist_d = dp.tile([T, BL, 2, 512], BF16)

            # ---- phase 1: precompute layer-0 input projection ----
            with tc.tile_pool(name="prepool", bufs=1) as pp, \
                 tc.tile_pool(name="prepsum", bufs=2, space="PSUM") as pps:
                xTf = pp.tile([128, T * BL], BF16)
                wih0T = pp.tile([128, G4], BF16)
                b0r = pp.tile([128, G4], BF16)
                nc.sync.dma_start(xTf[:], xT_d[:])
                nc.sync.dma_start(wih0T[:], wih0T_d[:])
                nc.sync.dma_start(b0r[:], b0r_d[:])
                for m in range(T * BL // 128):   # 32 token chunks (4 steps each)
                    for g in range(8):           # gate-column chunks
                        pspre = pps.tile([128, 512], F32, name="pspre", tag="pspre")
                        nc.tensor.matmul(pspre[:], xTf[:, ds(128 * m, 128)],
                                         wih0T[:, ds(512 * g, 512)],
                                         start=True, stop=True,
                                         skip_group_check=True)
                        evac = pp.tile([128, 512], BF16, name="evac", tag="evac",
                                       bufs=2)
                        bank, q = g // 4, g % 4
                        # add bias while evacuating (bias bf16, values tiny)
                        nc.vector.tensor_tensor(
                            evac[:], pspre[:], b0r[:, ds(512 * g, 512)], ADD)
                        for r in range(4):
                            nc.sync.dma_start(
                                xp_d[4 * m + r, bank, ds(32 * q, 32), :],
                                evac[ds(32 * r, 32), :])

            # ---- phase 2: recurrence ----
            with tc.tile_pool(name="wpool", bufs=1) as wp, \
                 tc.tile_pool(name="cpool", bufs=1) as cp, \
                 tc.tile_pool(name="gpsum", bufs=1, space="PSUM") as gp, \
                 tc.tile_pool(name="opsum", bufs=1, space="PSUM") as op:

                whh0T = wp.tile([128, KH * G4], BF16)
                wih1T = wp.tile([128, KH * G4], BF16)
                whh1T = wp.tile([128, KH * G4], BF16)
                b1q = wp.tile([128, 1024], BF16)
                h0T = wp.tile([128, KH * BL], BF16)
                h1T = wp.tile([128, KH * BL], BF16)
                c0 = wp.tile([128, 512], F32)
                c1 = wp.tile([128, 512], F32)
                xp = wp.tile([128, 1024], BF16)

                # shared cell temps (layer0 and layer1 alternate)
                sA = cp.tile([128, 512], F32)
                gB = cp.tile([128, 512], F32)
                ig = cp.tile([128, 512], F32)
                hb = cp.tile([128, 512], BF16)

                psA0 = gp.tile([128, 512], F32)
                psB0 = gp.tile([128, 512], F32)
                psA1 = gp.tile([128, 512], F32)
                psB1 = gp.tile([128, 512], F32)

                for k in range(KH):
                    nc.sync.dma_start(whh0T[:, ds(k * G4, G4)], whh0T_d[:, ds(k * G4, G4)])
                    nc.sync.dma_start(wih1T[:, ds(k * G4, G4)], wih1T_d[:, ds(k * G4, G4)])
                    nc.sync.dma_start(whh1T[:, ds(k * G4, G4)], whh1T_d[:, ds(k * G4, G4)])
                nc.sync.dma_start(b1q[:], b1q_d[:])
                nc.sync.dma_start(h0T[:], h0T_d[:])
                nc.sync.dma_start(h1T[:], h1T_d[:])
                nc.sync.dma_start(c0[64:128, :], c0_d[:])
                nc.sync.dma_start(c1[64:128, :], c1_d[:])

                colA = lambda q: 512 * q           # i[h0] i[h1] f[h0] f[h1]
                colB = lambda q: 2048 + 512 * q    # g[h0] g[h1] o[h0] o[h1]

                def hidden_rounds(ps, colf, hT_sb, wT_sb, start, stop):
                    for k in range(KH):
                        for q in range(4):
                            nc.tensor.matmul(
                                ps[ds(32 * q, 32), :], hT_sb[:, ds(32 * k, 32)],
                                wT_sb[:, ds(k * G4 + colf(q), 512)],
                                start=(start and k == 0),
                                stop=(stop and k == KH - 1),
                                tile_position=(0, 32 * q), skip_group_check=True)

                def cell(psA, psB, biasA, biasB, cst, l0):
                    # gates pre-activation: psum + (xp or b1q); ACT reads SBUF
                    nc.vector.tensor_tensor(sA[:], psA[:], biasA, ADD)
                    nc.vector.tensor_tensor(gB[:], psB[:], biasB, ADD)
                    nc.scalar.activation(sA[:], sA[:], AF.Sigmoid)
                    nc.scalar.activation(gB[0:64, :], gB[0:64, :], AF.Tanh)
                    nc.scalar.activation(gB[64:128, :], gB[64:128, :], AF.Sigmoid)
                    nc.vector.tensor_tensor(gB[0:64, :], sA[0:64, :], gB[0:64, :], MULT)
                    nc.sync.dma_start(ig[64:128, :], gB[0:64, :])  # partition shift
                    nc.vector.tensor_tensor(sA[64:128, :], sA[64:128, :], cst[64:128, :], MULT)
                    nc.vector.tensor_tensor(cst[64:128, :], ig[64:128, :], sA[64:128, :], ADD)
                    nc.scalar.activation(ig[64:128, :], cst[64:128, :], AF.Tanh)
                    nc.vector.tensor_tensor(hb[64:128, :], gB[64:128, :], ig[64:128, :], MULT)

                def transposes(hT_sb):
                    for cc in range(KH):
                        qt, m = cc // 4, cc % 4
                        nc.sync.dma_start(
                            hT_sb[:, ds(32 * cc, 32)],
                            hb[ds(64 + 32 * qt, 32), ds(128 * m, 128)],
                            transpose=True)

                def emit_step(tv, first, last):
                    nc.sync.dma_start(xp[:, 0:512], xp_d[tv, 0])
                    nc.sync.dma_start(xp[:, 512:1024], xp_d[tv, 1])
                    hidden_rounds(psA0, colA, h0T, whh0T, True, True)
                    hidden_rounds(psB0, colB, h0T, whh0T, True, True)
                    # layer 1 hidden part (independent of cell0 -> fills PE)
                    hidden_rounds(psA1, colA, h1T, whh1T, True, False)
                    hidden_rounds(psB1, colB, h1T, whh1T, True, False)
                    cell(psA0, psB0, xp[:, 0:512], xp[:, 512:1024], c0, True)
                    transposes(h0T)
                    # layer 1 input part (needs new h0T)
                    hidden_rounds(psA1, colA, h0T, wih1T, False, True)
                    hidden_rounds(psB1, colB, h0T, wih1T, False, True)
                    cell(psA1, psB1, b1q[:, 0:512], b1q[:, 512:1024], c1, False)
                    transposes(h1T)
                    # h1 history for the output projection (A-row flat layout)
                    for qt in range(2):
                        nc.sync.dma_start(hist_d[tv, :, qt, :],
                                          hb[ds(64 + 32 * qt, 32), :])

                emit_step(0, True, False)
                if _DUMP:
                    nc.sync.dma_start(dmp["xp"][:], xp[:])
                with tc.For_i(1, T - 1, 1,
                              hint_engines=(mybir.EngineType.PE,)) as tv:
                    emit_step(tv, False, False)
                emit_step(T - 1, False, True)

                if _DUMP:
                    nc.sync.dma_start(dmp["h0T"][:], h0T[:])
                    nc.sync.dma_start(dmp["h1T"][:], h1T[:])
                    nc.sync.dma_start(dmp["c0"][:], c0[64:128, :])
                    nc.sync.dma_start(dmp["c1"][:], c1[64:128, :])

            # ---- phase 3: output projection  partial[t,o] = A @ WoutSlice^T ----
            with tc.tile_pool(name="fpool", bufs=6) as fp, \
                 tc.tile_pool(name="fpsum", bufs=1, space="PSUM") as fps:
                outp = fps.tile([128, O], F32)
                for kp in range(256):
                    b, sub = kp // 8, kp % 8
                    qt, jlo = sub // 4, 128 * (sub % 4)
                    ofT = fp.tile([128, 128], BF16, name="ofT", tag="ofT")
                    nc.sync.dma_start(ofT[:], hist_d[:, b, qt, ds(jlo, 128)],
                                      transpose=True)
                    wos = fp.tile([128, O], BF16, name="wos", tag="wos")
                    nc.sync.dma_start(wos[:], woS_d[kp])
                    nc.tensor.matmul(outp[:], ofT[:], wos[:],
                                     start=(kp == 0), stop=(kp == 255),
                                     skip_group_check=True)
                oev = fp.tile([128, O], F32)
                nc.vector.tensor_copy(oev[:], outp[:])
                nc.sync.dma_start(out_d[:], oev[:])

    nc.compile()
    return nc


def _quad_bias(b):
    # [4096] -> [128, 1024]: partition 32q+b rows replicate bias of chunk q
    # cols 0:512 = bank A chunk (i/f), cols 512:1024 = bank B chunk (g/o)
    bA = b[0:2048].reshape(4, 512)      # i[h0] i[h1] f[h0] f[h1]
    bB = b[2048:4096].reshape(4, 512)   # g[h0] g[h1] o[h0] o[h1]
    out = np.zeros((128, 1024), np.float32)
    for q in range(4):
        out[32 * q:32 * q + 32, 0:512] = bA[q]
        out[32 * q:32 * q + 32, 512:1024] = bB[q]
    return out.astype(ml_dtypes.bfloat16)


def _prep(inputs):
    bf = ml_dtypes.bfloat16

    def wT(w):  # [4H, K] -> [128, KH*4H] chunked transpose
        kk = w.shape[1] // 128
        return np.ascontiguousarray(
            w.T.reshape(kk, 128, G4).transpose(1, 0, 2).reshape(128, kk * G4)
        ).astype(bf)

    shared = {
        "whh0T": wT(np.asarray(inputs["Whh0"], np.float32)),
        "wih1T": wT(np.asarray(inputs["Wih1"], np.float32)),
        "whh1T": wT(np.asarray(inputs["Whh1"], np.float32)),
        "wih0T": np.ascontiguousarray(np.asarray(inputs["Wih0"], np.float32).T).astype(bf),
        "b0r": np.ascontiguousarray(np.broadcast_to(
            (np.asarray(inputs["bih0"], np.float32)
             + np.asarray(inputs["bhh0"], np.float32))[None, :],
            (128, G4))).astype(ml_dtypes.bfloat16),
        "b1q": _quad_bias(np.asarray(inputs["bih1"], np.float32)
                          + np.asarray(inputs["bhh1"], np.float32)),
    }
    WoT = np.asarray(inputs["Wout"], np.float32).T  # [T*H, O]
    xr = np.asarray(inputs["batch"], np.float32).reshape(T, B, F)
    in_maps = []
    for c in range(NCORES):
        sl = slice(BL * c, BL * (c + 1))
        m = dict(shared)
        m["woS"] = np.ascontiguousarray(
            WoT[32768 * (c % 4):32768 * (c % 4) + 32768].reshape(256, 128, O)
        ).astype(bf)
        m["xT"] = np.ascontiguousarray(
            xr[:, sl, :].transpose(2, 0, 1).reshape(F, T * BL)).astype(bf)
        for nm, hsrc in (("h0T", "h00"), ("h1T", "h01")):
            h = np.asarray(inputs[hsrc], np.float32)[sl]  # [32, 1024]
            m[nm] = np.ascontiguousarray(
                h.T.reshape(KH, 128, BL).transpose(1, 0, 2).reshape(128, KH * BL)
            ).astype(bf)
        for nm, csrc in (("c0", "c00"), ("c1", "c01")):
            cc = np.asarray(inputs[csrc], np.float32)[sl]  # [32, 1024]
            m[nm] = np.ascontiguousarray(
                cc.reshape(BL, 2, 512).transpose(1, 0, 2).reshape(64, 512))
        in_maps.append(m)
    return in_maps


def kernel(**inputs):
    from concourse import bass_utils

    if "nc" not in _cache:
        _cache["nc"] = _build()
    nc = _cache["nc"]
    in_maps = _prep(inputs)
    r = bass_utils.run_bass_kernel_spmd(nc, in_maps, core_ids=list(range(NCORES)))
    parts = np.stack([r.results[c]["out"] for c in range(NCORES)])  # [8, T, O]
    bout = np.asarray(inputs["bout"], np.float32)
    out = np.empty((B, O), np.float32)
    half0 = parts[0:4].sum(axis=0) + bout   # rows 2t
    half1 = parts[4:8].sum(axis=0) + bout   # rows 2t+1
    out[0::2] = half0
    out[1::2] = half1
    return out
